# revision 3
# baseline (speedup 1.0000x reference)
"""Multi-head attention (B=2, S=2048, D=1024, H=16, causal) on 8 Trainium2
NeuronCores via Bass/Tile.

Sharding: core c -> batch c//4, heads [4*(c%4), 4*(c%4)+4)  (DP over batch x
TP over heads).  QKV weights column-parallel, O row-parallel; the 4 partial
[S, D] outputs per batch are summed on the host (gather step), bias bo added
there too.

Per-core dataflow (bf16 matmuls, fp32 PSUM accumulation):
  - host supplies x.T [D, S] per batch (so the d_in contraction dim lands on
    SBUF partitions), plus pre-swizzled weight blocks.  DMA order follows
    compute order (wk+xk first) so the PE starts ~3us in.
  - Q/K proj -> qT/kT [dk_c=256, S] (head-major, 2 chunks of 128 = 2 heads).
  - V proj  -> natural [S, 260] layout: per head 64 cols of V plus a ones
    column (written via the bias row) for the flash-style softmax denominator.
  - scores computed transposed: sT[k, q] = kT.T @ qT per head; on the 4
    diagonal 128-blocks of each 512 q-chunk the matmul/exp/attn@V are trimmed
    to columns >= 128*jr (causal), and a single [128,128] triangular keep
    mask handles the remaining partial block; strictly-upper blocks skipped.
  - exp on ScalarE reads PSUM directly via a 3-D AP covering both k-blocks of
    a group; attn@V: outT[65, q] += V'[k,65].T @ expT[k,q]; row 64 accumulates
    the softmax denominator.  Normalize = reciprocal on lane 64 (exp(-ln x))
    + PE outer-product broadcast + Pool-engine multiply.
  - attn-out stored as head-PAIR tiles [128, S] so the O projection contracts
    over 128 partitions (2 heads x 64 dk) per matmul: half the matmuls of the
    per-head variant.  PSUM -> f16 SBUF (GpSimd copy) -> DRAM f16 partials.
  - attention groups are software-pipelined one group deep (attn@V of group
    g-1 issues after scores of group g) and the bc/oproj tail of the previous
    q-chunk fills the PE while the last group's exp drains.
"""

import os
import sys
import types

import numpy as np

B, S, D, H = 2, 2048, 1024, 16
DK = D // H  # 64
N_CORES = 8
HPC = 4  # heads per core
SCALE = 1.0 / np.sqrt(np.float32(DK))  # folded into Wq/bq on host

QC = 512  # query block (free dim of scores matmuls)
NQC = S // QC  # 4
KC = 128  # key block (partition dim of transposed scores)
GK = 2  # key blocks per exp group -> scores psum tile [128, GK, QC]


def _install_ntff_hook():
    """The image's antenv lacks axon_hooks; register the NTFF profile hook
    ourselves so run_bass_kernel_spmd(trace=True) works."""
    if "antenv.axon_hooks" in sys.modules:
        return
    try:
        mod = types.ModuleType("antenv.axon_hooks")
        state = {"hook": None}
        mod.set_axon_ntff_profile_hook = lambda h: state.__setitem__("hook", h)
        mod.get_axon_ntff_profile_hook = lambda: state["hook"]
        sys.modules["antenv.axon_hooks"] = mod
        from trn_agent_boot.trn_boot import _ntff_profile_via_ctypes

        mod.set_axon_ntff_profile_hook(
            _ntff_profile_via_ctypes("/opt/axon/libaxon_pjrt.so")
        )
    except Exception:
        sys.modules.pop("antenv.axon_hooks", None)


def _split_multi_waits(nc):
    """This walrus build accepts at most ONE sem wait per instruction; Tile
    packs several.  Split extras into preceding single-wait NOPs on the same
    engine (equivalent semantics: the engine blocks on them in order)."""
    import bass_rust

    cnt = 0
    for bbw in nc.main_func.blocks:
        bb = bbw.bb if hasattr(bbw, "bb") else bbw
        out = []
        changed = False
        for ins in bb.instructions:
            si = ins.sync_info
            if si is not None and len(si.on_wait) > 1:
                changed = True
                waits = list(si.on_wait)
                for w in waits[:-1]:
                    cnt += 1
                    nop = bass_rust.InstNoOp(name=f"I-wsp{cnt}", ins=[], outs=[])
                    nop.engine = ins.engine
                    nop.sync_info = bass_rust.SyncInfo(on_wait=[w], on_update=[])
                    out.append(nop)
                si.on_wait = [waits[-1]]
                ins.sync_info = si
            out.append(ins)
        if changed:
            bb.instructions = out
    return cnt


def _build_nc(split=True, phase=5):
    from contextlib import ExitStack

    import concourse.bass as bass
    import concourse.tile as tile
    from concourse import mybir

    bf16 = mybir.dt.bfloat16
    f16 = mybir.dt.float16
    f32 = mybir.dt.float32

    nc = bass.Bass()
    xqT = nc.declare_dram_parameter("xqT", [D, S], bf16, isOutput=False)
    xkT = nc.declare_dram_parameter("xkT", [D, S], bf16, isOutput=False)
    xvT = nc.declare_dram_parameter("xvT", [D, S], bf16, isOutput=False)
    wq = nc.declare_dram_parameter("wq", [128, 8 * 256], bf16, isOutput=False)
    wk = nc.declare_dram_parameter("wk", [128, 8 * 256], bf16, isOutput=False)
    wv = nc.declare_dram_parameter("wv", [128, 8 * 260], bf16, isOutput=False)
    wo = nc.declare_dram_parameter("wo", [128, 2 * 1024], bf16, isOutput=False)
    bq = nc.declare_dram_parameter("bq", [128, 2], f32, isOutput=False)
    bk = nc.declare_dram_parameter("bk", [128, 2], f32, isOutput=False)
    bvp = nc.declare_dram_parameter("bvp", [1, 260], f32, isOutput=False)
    cmask = nc.declare_dram_parameter("cmask", [128, 128], bf16, isOutput=False)
    outp = nc.declare_dram_parameter("outp", [S, D], f16, isOutput=True)

    with tile.TileContext(nc) as tc, ExitStack() as ctx:
        consts = ctx.enter_context(tc.tile_pool(name="consts", bufs=1))
        xs = ctx.enter_context(tc.tile_pool(name="xs", bufs=10))
        acts = ctx.enter_context(tc.tile_pool(name="acts", bufs=1))
        exps = ctx.enter_context(tc.tile_pool(name="exps", bufs=6))
        rcps = ctx.enter_context(tc.tile_pool(name="rcps", bufs=4))
        osb = ctx.enter_context(tc.tile_pool(name="osb", bufs=4))
        ps_small = ctx.enter_context(
            tc.tile_pool(name="ps_small", bufs=2, space="PSUM")
        )
        ps_sc = ctx.enter_context(tc.tile_pool(name="ps_sc", bufs=2, space="PSUM"))
        ps_av = ctx.enter_context(tc.tile_pool(name="ps_av", bufs=2, space="PSUM"))

        # ---- persistent activation tiles ----
        qt = [acts.tile([128, S], bf16, name=f"qt{m}", tag=f"qt{m}") for m in range(2)]
        kt = [acts.tile([128, S], bf16, name=f"kt{m}", tag=f"kt{m}") for m in range(2)]
        vh_sb = acts.tile([128, 16, 260], bf16, name="vh", tag="vh")
        # attn-out as head PAIRS [2 heads x 64 dk = 128 partitions, S]
        outT = [
            acts.tile([128, S], bf16, name=f"outT{p}", tag=f"outT{p}")
            for p in range(2)
        ]

        # ---- constants (DMA order == consume order: K first) ----
        wk_sb = consts.tile([128, 8 * 256], bf16)
        nc.sync.dma_start(out=wk_sb[:], in_=wk[:])
        bk_sb = consts.tile([128, 2], f32)
        nc.sync.dma_start(out=bk_sb[:], in_=bk[:])

        def emit_kq_proj(src_, wsb, bsb, dst, dma_only=None):
            xt = {}
            for half in range(2):
                for dc in range(8):
                    t = xs.tile([128, S // 2], bf16, name="xt", tag="xt")
                    nc.sync.dma_start(
                        out=t[:],
                        in_=src_[
                            dc * 128:(dc + 1) * 128,
                            half * 1024:(half + 1) * 1024,
                        ],
                    )
                    xt[(dc, half)] = t
            for half in range(2):
                for m in range(2):
                    for scq in range(2):
                        sc = half * 2 + scq
                        ps = ps_small.tile([128, 512], f32, name="ps", tag="ps")
                        for dc in range(8):
                            nc.tensor.matmul(
                                ps[:],
                                lhsT=wsb[
                                    :, dc * 256 + m * 128: dc * 256 + (m + 1) * 128
                                ],
                                rhs=xt[(dc, half)][:, scq * 512:(scq + 1) * 512],
                                start=(dc == 0),
                                stop=(dc == 7),
                            )
                        # copy+bias+downcast: out = psum + b (per-partition)
                        nc.vector.tensor_scalar_add(
                            dst[m][:, sc * 512:(sc + 1) * 512],
                            ps[:],
                            bsb[:, m:m + 1],
                        )

        def emit_v_proj(wv_sb, bvp_sb):
            xt = {}
            for half in range(2):
                for dc in range(8):
                    t = xs.tile([128, S // 2], bf16, name="xt", tag="xt")
                    nc.sync.dma_start(
                        out=t[:],
                        in_=xvT[
                            dc * 128:(dc + 1) * 128, half * 1024:(half + 1) * 1024
                        ],
                    )
                    xt[(dc, half)] = t
            for st in range(16):
                ps = ps_small.tile([128, 512], f32, name="ps", tag="ps")
                for dc in range(8):
                    nc.tensor.matmul(
                        ps[:, :260],
                        lhsT=xt[(dc, st // 8)][:, (st % 8) * 128:(st % 8 + 1) * 128],
                        rhs=wv_sb[:, dc * 260:(dc + 1) * 260],
                        start=(dc == 0),
                        stop=(dc == 7),
                    )
                # +bias (varies along free dim; bvp_sb is the DMA-broadcast
                # row), writes the ones column too (bvp has 1.0 at h*65+64).
                nc.vector.tensor_add(vh_sb[:, st, :], ps[:, :260], bvp_sb[:])

        if phase >= 1:
            emit_kq_proj(xkT, wk_sb, bk_sb, kt)
        # remaining constants, in consume order
        wq_sb = consts.tile([128, 8 * 256], bf16, name="wq_sb")
        nc.sync.dma_start(out=wq_sb[:], in_=wq[:])
        bq_sb = consts.tile([128, 2], f32, name="bq_sb")
        nc.sync.dma_start(out=bq_sb[:], in_=bq[:])
        cm_sb = consts.tile([128, 128], bf16, name="cm_sb")
        nc.sync.dma_start(out=cm_sb[:], in_=cmask[:])
        if phase >= 1:
            emit_kq_proj(xqT, wq_sb, bq_sb, qt)
        wv_sb = consts.tile([128, 8 * 260], bf16, name="wv_sb")
        nc.sync.dma_start(out=wv_sb[:], in_=wv[:])
        bvp_sb = consts.tile([128, 260], f32, name="bvp_sb")
        nc.sync.dma_start(out=bvp_sb[:], in_=bvp[:].to_broadcast((128, 260)))
        if phase >= 2:
            emit_v_proj(wv_sb, bvp_sb)
        wo_sb = consts.tile([128, 2 * 1024], bf16, name="wo_sb")
        nc.sync.dma_start(out=wo_sb[:], in_=wo[:])
        ones_sb = consts.tile([65, 64], bf16)
        nc.vector.memset(ones_sb[:], 1.0)

        # ---- attention ----
        if phase >= 3:
            def trim_c0(qc, kc):
                """Causal column trim for kc-block within q-chunk qc: the
                first 128*jr columns of a diagonal block are fully masked."""
                jr = kc - 4 * qc
                return 128 * jr if jr >= 0 else 0

            def emit_scores_exp(qc, pair, g, exg):
                """Scores matmuls + exp for group g (both heads of pair)."""
                heads = (2 * pair, 2 * pair + 1)
                dg = g - GK * qc  # diagonal subgroup index (>=0 on diagonal)
                c0m = 256 * dg if dg >= 0 else 0  # min trim over the 2 blocks
                for h in heads:
                    hr = slice(64 * (h % 2), 64 * (h % 2) + 64)
                    pss = ps_sc.tile([128, GK, QC], f32, name="pss", tag="pss")
                    for j in range(GK):
                        kc = GK * g + j
                        c0 = trim_c0(qc, kc)
                        nc.tensor.matmul(
                            pss[:, j, c0:],
                            lhsT=kt[pair][hr, kc * 128:(kc + 1) * 128],
                            rhs=qt[pair][hr, qc * QC + c0:(qc + 1) * QC],
                            start=True,
                            stop=True,
                        )
                    ex = exps.tile([128, GK, QC], bf16, name="ex", tag="ex")
                    nc.scalar.activation(
                        ex[:, :, c0m:],
                        pss[:, :, c0m:],
                        mybir.ActivationFunctionType.Exp,
                    )
                    if dg >= 0:
                        # triangular 128-block of each diagonal kc: keep mask
                        for j in range(GK):
                            c0 = trim_c0(qc, GK * g + j)
                            nc.vector.tensor_mul(
                                ex[:, j, c0:c0 + 128],
                                ex[:, j, c0:c0 + 128],
                                cm_sb[:],
                            )
                    exg[h] = ex

            def emit_attnv(qc, pair, g, po, last_kc, exg):
                heads = (2 * pair, 2 * pair + 1)
                for h in heads:
                    for j in range(GK):
                        kc = GK * g + j
                        c0 = trim_c0(qc, kc)
                        nc.tensor.matmul(
                            po[h][:, c0:],
                            lhsT=vh_sb[:, kc, h * 65:(h + 1) * 65],
                            rhs=exg[h][:, j, c0:],
                            start=(kc == 0),
                            stop=(kc == last_kc),
                            skip_group_check=True,
                        )

            def emit_pair_tail(qc, pair, po, cur_posb, cur_rcp):
                # denominator reciprocal on ScalarE (exp(-ln x); both funcs in
                # the natural_log_exp table set) + stage attn-out to SBUF bf16
                # so the po PSUM bank frees immediately.
                for h in (2 * pair, 2 * pair + 1):
                    lg = rcps.tile([65, 512], f32, name="lg", tag="lg", bufs=4)
                    nc.scalar.activation(
                        lg[64:65, :],
                        po[h][64:65, :],
                        mybir.ActivationFunctionType.Ln,
                    )
                    rcp = rcps.tile([65, 512], bf16, name="rcp", tag="rcp", bufs=8)
                    nc.scalar.activation(
                        rcp[64:65, :],
                        lg[64:65, :],
                        mybir.ActivationFunctionType.Exp,
                        scale=-1.0,
                    )
                    posb = rcps.tile([64, 512], bf16, name="posb", tag="posb", bufs=8)
                    with nc.allow_low_precision(reason="attn-out staged bf16"):
                        nc.vector.tensor_copy(posb[:, :], po[h][0:64, :])
                    cur_posb[h] = posb
                    cur_rcp[h] = rcp

            def emit_bc(qc, posb, rcp):
                for h in range(HPC):
                    bc = ps_small.tile([128, 512], f32, name="ps", tag="ps")
                    nc.tensor.matmul(
                        bc[0:64, :],
                        lhsT=ones_sb[64:65, :],
                        rhs=rcp[h][64:65, :],
                        start=True,
                        stop=True,
                    )
                    # DVE can't take two PSUM operands; stage via SBUF, then
                    # the normalize multiply runs on the otherwise-idle GpSimd.
                    bcs = rcps.tile([64, 512], bf16, name="bcs", tag="bcs", bufs=8)
                    nc.vector.tensor_copy(bcs[:, :], bc[0:64, :])
                    nc.gpsimd.tensor_mul(
                        outT[h // 2][64 * (h % 2):64 * (h % 2) + 64,
                                     qc * QC:(qc + 1) * QC],
                        posb[h][:, :],
                        bcs[:, :],
                    )

            def emit_oproj(qc):
                for sti in range(4):
                    st = qc * 4 + sti
                    for ns in range(2):
                        ps = ps_small.tile([128, 512], f32, name="ps", tag="ps")
                        for hp in range(2):
                            nc.tensor.matmul(
                                ps[:],
                                lhsT=outT[hp][:, st * 128:(st + 1) * 128],
                                rhs=wo_sb[
                                    :, hp * 1024 + ns * 512: hp * 1024 + (ns + 1) * 512
                                ],
                                start=(hp == 0),
                                stop=(hp == 1),
                            )
                        ot = osb.tile([128, 512], f16, name="ot", tag="ot")
                        with nc.allow_low_precision(reason="f16 partials"):
                            nc.vector.tensor_copy(ot[:], ps[:])
                        nc.sync.dma_start(
                            out=outp[
                                st * 128:(st + 1) * 128, ns * 512:(ns + 1) * 512
                            ],
                            in_=ot[:],
                        )

            # Per (qc, pair): groups are software-pipelined one deep; the
            # bc/oproj tail of the previous qc fills the PE while the last
            # group's exp drains.
            pending = None  # (qc, posb{h}, rcp{h})
            for qc in range(NQC):
                cur_posb = {}
                cur_rcp = {}
                ngroups = GK * (qc + 1)
                last_kc = 4 * qc + 3
                for pair in range(2):
                    po = {}
                    for h in (2 * pair, 2 * pair + 1):
                        po[h] = ps_av.tile([65, 512], f32, name="po", tag="po")
                    prev = None  # (g, exg)
                    for g in range(ngroups):
                        exg = {}
                        emit_scores_exp(qc, pair, g, exg)
                        if g == ngroups - 1 and pending is not None:
                            # PE filler while exp of the last group drains
                            if pair == 0 and phase >= 4:
                                emit_bc(pending[0], pending[1], pending[2])
                            elif pair == 1 and phase >= 5:
                                emit_oproj(pending[0])
                        if prev is not None:
                            emit_attnv(qc, pair, prev[0], po, last_kc, prev[1])
                        prev = (g, exg)
                    emit_attnv(qc, pair, prev[0], po, last_kc, prev[1])
                    if pending is not None and qc == 0:
                        # no groups to hide behind at qc=0: emit tail work now
                        pass
                    emit_pair_tail(qc, pair, po, cur_posb, cur_rcp)
                pending = (qc, cur_posb, cur_rcp)
            if pending is not None and phase >= 4:
                emit_bc(pending[0], pending[1], pending[2])
                if phase >= 5:
                    emit_oproj(pending[0])

        if phase < 5:
            ot = osb.tile([128, 512], f16, name="ot", tag="ot")
            nc.vector.memset(ot[:], 0.0)
            nc.sync.dma_start(out=outp[0:128, 0:512], in_=ot[:])

    if split:
        _split_multi_waits(nc)
    return nc


_NC_CACHE = None


def _get_nc():
    global _NC_CACHE
    if _NC_CACHE is None:
        _NC_CACHE = _build_nc()
    return _NC_CACHE


def _swizzle_w(wT, block):
    """wT [D, C] -> [128, 8*C] so that out[p, dc*C + j] = wT[dc*128 + p, j]."""
    dcs = wT.shape[0] // 128
    return np.ascontiguousarray(
        wT.reshape(dcs, 128, wT.shape[1]).transpose(1, 0, 2).reshape(128, -1)
    )


def _np_reference(q, k, v, mask, Wq, bq, Wk, bk, Wv, bv, Wo, bo):
    def split_heads(x):
        b, s, _ = x.shape
        return x.reshape(b, s, H, DK).transpose(0, 2, 1, 3)

    qh = split_heads(q @ Wq.T + bq)
    kh = split_heads(k @ Wk.T + bk)
    vh = split_heads(v @ Wv.T + bv)
    scores = np.einsum("bhqd,bhkd->bhqk", qh, kh) / np.sqrt(np.float32(DK))
    scores = np.where(mask, np.float32(-1e9), scores)
    scores = scores - scores.max(axis=-1, keepdims=True)
    e = np.exp(scores)
    attn = e / e.sum(axis=-1, keepdims=True)
    out = np.einsum("bhqk,bhkd->bhqd", attn, vh)
    out = out.transpose(0, 2, 1, 3).reshape(q.shape[0], -1, D)
    return (out @ Wo.T + bo).astype(np.float32)


def kernel(q, k, v, mask, Wq, bq, Wk, bk, Wv, bv, Wo, bo):
    import ml_dtypes

    bf16 = ml_dtypes.bfloat16

    q = np.asarray(q, np.float32)
    k = np.asarray(k, np.float32)
    v = np.asarray(v, np.float32)
    mask = np.asarray(mask, bool)
    Wq = np.asarray(Wq, np.float32)
    bq = np.asarray(bq, np.float32)
    Wk = np.asarray(Wk, np.float32)
    bk = np.asarray(bk, np.float32)
    Wv = np.asarray(Wv, np.float32)
    bv = np.asarray(bv, np.float32)
    Wo = np.asarray(Wo, np.float32)
    bo = np.asarray(bo, np.float32)

    causal = np.triu(np.ones((S, S), dtype=bool), k=1)
    if not np.array_equal(mask.reshape(S, S), causal):
        return _np_reference(q, k, v, mask, Wq, bq, Wk, bk, Wv, bv, Wo, bo)

    _install_ntff_hook()
    from concourse.bass_utils import run_bass_kernel_spmd

    nc = _get_nc()

    # triangular keep-mask for the partial 128x128 diagonal blocks:
    # keep iff kk <= qq
    kk = np.arange(128)[:, None]
    qq = np.arange(128)[None, :]
    cm = (kk <= qq).astype(bf16)  # [128, 128]

    xT = {}
    for name, x in (("q", q), ("k", k), ("v", v)):
        xT[name] = [np.ascontiguousarray(x[b].T).astype(bf16) for b in range(B)]

    in_maps = []
    for c in range(N_CORES):
        b = c // 4
        g = c % 4
        hs = slice(g * HPC * DK, (g + 1) * HPC * DK)  # 256 rows of W, cols of Wo
        wq_c = _swizzle_w((SCALE * Wq[hs]).T.astype(bf16), 256)
        wk_c = _swizzle_w(Wk[hs].T.astype(bf16), 256)
        # V' with a zero weight column at h*65+64 (ones come via bias row)
        wvT = Wv[hs].T  # [1024, 256]
        wvp = np.zeros((D, 260), np.float32)
        for h in range(HPC):
            wvp[:, h * 65:h * 65 + 64] = wvT[:, h * 64:(h + 1) * 64]
        wv_c = _swizzle_w(wvp.astype(bf16), 260)
        # wo: (Wo.T)[hs, :] [256, 1024] -> head-pair blocks [128, 2*1024]
        woT = np.ascontiguousarray(Wo[:, hs].T)
        wo_c = np.ascontiguousarray(
            woT.reshape(2, 128, 1024).transpose(1, 0, 2).reshape(128, 2048)
        ).astype(bf16)
        bq_c = np.ascontiguousarray(
            (SCALE * bq[hs]).reshape(2, 128).T.astype(np.float32)
        )
        bk_c = np.ascontiguousarray(bk[hs].reshape(2, 128).T.astype(np.float32))
        bvp_c = np.zeros((1, 260), np.float32)
        for h in range(HPC):
            bvp_c[0, h * 65:h * 65 + 64] = bv[hs][h * 64:(h + 1) * 64]
            bvp_c[0, h * 65 + 64] = 1.0
        in_maps.append(
            {
                "xqT": xT["q"][b],
                "xkT": xT["k"][b],
                "xvT": xT["v"][b],
                "wq": wq_c,
                "wk": wk_c,
                "wv": wv_c,
                "wo": wo_c,
                "bq": bq_c,
                "bk": bk_c,
                "bvp": bvp_c,
                "cmask": cm,
            }
        )

    trace = bool(os.environ.get("BASSMHA_TRACE"))
    res = run_bass_kernel_spmd(nc, in_maps, list(range(N_CORES)), trace=trace)
    kernel._last_exec_ns = res.exec_time_ns
    kernel._last_mean_exec_ns = res.mean_exec_time_ns

    out = np.zeros((B, S, D), np.float64)
    for c in range(N_CORES):
        out[c // 4] += res.results[c]["outp"].astype(np.float64)
    out += bo.astype(np.float64)
    return out.astype(np.float32)


# revision 15
# speedup vs baseline: 1.0946x; 1.0946x over previous
"""Multi-head attention (B=2, S=2048, D=1024, H=16, causal) on 8 Trainium2
NeuronCores via Bass/Tile.

Sharding: core c -> batch c//4, heads [4*(c%4), 4*(c%4)+4)  (DP over batch x
TP over heads).  QKV weights column-parallel, O row-parallel; the 4 partial
[S, D] outputs per batch are summed on the host (gather step), bias bo added
there too.

Per-core dataflow (bf16 matmuls, fp32 PSUM accumulation):
  - host supplies x.T [D, S] per batch (so the d_in contraction dim lands on
    SBUF partitions), plus pre-swizzled weight blocks.  DMA order follows
    compute order (wk+xk first) so the PE starts ~3us in.
  - Q/K proj -> qT/kT [dk_c=256, S] (head-major, 2 chunks of 128 = 2 heads).
  - V proj  -> natural [S, 260] layout: per head 64 cols of V plus a ones
    column (written via the bias row) for the flash-style softmax denominator.
  - scores computed transposed: sT[k, q] = kT.T @ qT per head; on the 4
    diagonal 128-blocks of each 512 q-chunk the matmul/exp/attn@V are trimmed
    to columns >= 128*jr (causal), and a single [128,128] triangular keep
    mask handles the remaining partial block; strictly-upper blocks skipped.
  - exp on ScalarE reads PSUM directly via a 3-D AP covering both k-blocks of
    a group; attn@V: outT[65, q] += V'[k,65].T @ expT[k,q]; row 64 accumulates
    the softmax denominator.  Normalize = reciprocal on lane 64 (exp(-ln x))
    + PE outer-product broadcast + Pool-engine multiply.
  - attn-out stored as head-PAIR tiles [128, S] so the O projection contracts
    over 128 partitions (2 heads x 64 dk) per matmul: half the matmuls of the
    per-head variant.  PSUM -> f16 SBUF (GpSimd copy) -> DRAM f16 partials.
  - attention groups are software-pipelined one group deep (attn@V of group
    g-1 issues after scores of group g) and the bc/oproj tail of the previous
    q-chunk fills the PE while the last group's exp drains.
"""

import os
import sys
import types

import numpy as np

B, S, D, H = 2, 2048, 1024, 16
DK = D // H  # 64
N_CORES = 8
HPC = 4  # heads per core
SCALE = 1.0 / np.sqrt(np.float32(DK))  # folded into Wq/bq on host

QC = 512  # query block (free dim of scores matmuls)
NQC = S // QC  # 4
KC = 128  # key block (partition dim of transposed scores)
GK = 2  # key blocks per exp group -> scores psum tile [128, GK, QC]


def _install_ntff_hook():
    """The image's antenv lacks axon_hooks; register the NTFF profile hook
    ourselves so run_bass_kernel_spmd(trace=True) works."""
    if "antenv.axon_hooks" in sys.modules:
        return
    try:
        mod = types.ModuleType("antenv.axon_hooks")
        state = {"hook": None}
        mod.set_axon_ntff_profile_hook = lambda h: state.__setitem__("hook", h)
        mod.get_axon_ntff_profile_hook = lambda: state["hook"]
        sys.modules["antenv.axon_hooks"] = mod
        from trn_agent_boot.trn_boot import _ntff_profile_via_ctypes

        mod.set_axon_ntff_profile_hook(
            _ntff_profile_via_ctypes("/opt/axon/libaxon_pjrt.so")
        )
    except Exception:
        sys.modules.pop("antenv.axon_hooks", None)


def _split_multi_waits(nc):
    """This walrus build accepts at most ONE sem wait per instruction; Tile
    packs several.  Split extras into preceding single-wait NOPs on the same
    engine (equivalent semantics: the engine blocks on them in order)."""
    import bass_rust

    cnt = 0
    for bbw in nc.main_func.blocks:
        bb = bbw.bb if hasattr(bbw, "bb") else bbw
        out = []
        changed = False
        for ins in bb.instructions:
            si = ins.sync_info
            if si is not None and len(si.on_wait) > 1:
                changed = True
                waits = list(si.on_wait)
                for w in waits[:-1]:
                    cnt += 1
                    nop = bass_rust.InstNoOp(name=f"I-wsp{cnt}", ins=[], outs=[])
                    nop.engine = ins.engine
                    nop.sync_info = bass_rust.SyncInfo(on_wait=[w], on_update=[])
                    out.append(nop)
                si.on_wait = [waits[-1]]
                ins.sync_info = si
            out.append(ins)
        if changed:
            bb.instructions = out
    return cnt


def _build_nc(split=True, phase=5):
    from contextlib import ExitStack

    import concourse.bass as bass
    import concourse.tile as tile
    from concourse import mybir

    bf16 = mybir.dt.bfloat16
    f16 = mybir.dt.float16
    f32 = mybir.dt.float32

    nc = bass.Bass()
    # x.T in dc-swizzled layout [p, dc, s]: per-partition 16KB contiguous in
    # DRAM so each transfer runs at full DMA bandwidth (two halves per tensor
    # so compute can start after the first half lands).
    xq_h = [
        nc.declare_dram_parameter(f"xq{i}", [128, 8, 1024], bf16, isOutput=False)
        for i in range(2)
    ]
    xk_h = [
        nc.declare_dram_parameter(f"xk{i}", [128, 8, 1024], bf16, isOutput=False)
        for i in range(2)
    ]
    xv_h = [
        nc.declare_dram_parameter(f"xv{i}", [128, 8, 1024], bf16, isOutput=False)
        for i in range(2)
    ]
    wq = nc.declare_dram_parameter("wq", [128, 8 * 256], bf16, isOutput=False)
    wk = nc.declare_dram_parameter("wk", [128, 8 * 256], bf16, isOutput=False)
    wv = nc.declare_dram_parameter("wv", [128, 8 * 260], bf16, isOutput=False)
    wo = nc.declare_dram_parameter("wo", [128, 2 * 1024], bf16, isOutput=False)
    bq = nc.declare_dram_parameter("bq", [128, 2], f32, isOutput=False)
    bk = nc.declare_dram_parameter("bk", [128, 2], f32, isOutput=False)
    bvp = nc.declare_dram_parameter("bvp", [1, 260], f32, isOutput=False)
    cmask = nc.declare_dram_parameter("cmask", [128, 128], bf16, isOutput=False)
    outp = nc.declare_dram_parameter("outp", [S, D], f16, isOutput=True)

    with tile.TileContext(nc) as tc, ExitStack() as ctx:
        consts = ctx.enter_context(tc.tile_pool(name="consts", bufs=1))
        xs = ctx.enter_context(tc.tile_pool(name="xs", bufs=4))
        acts = ctx.enter_context(tc.tile_pool(name="acts", bufs=1))
        exps = ctx.enter_context(tc.tile_pool(name="exps", bufs=6))
        rcps = ctx.enter_context(tc.tile_pool(name="rcps", bufs=4))
        osb = ctx.enter_context(tc.tile_pool(name="osb", bufs=4))
        ps_small = ctx.enter_context(
            tc.tile_pool(name="ps_small", bufs=2, space="PSUM")
        )
        ps_sc = ctx.enter_context(tc.tile_pool(name="ps_sc", bufs=2, space="PSUM"))
        ps_av = ctx.enter_context(tc.tile_pool(name="ps_av", bufs=2, space="PSUM"))

        # ---- persistent activation tiles ----
        qt = [acts.tile([128, S], bf16, name=f"qt{m}", tag=f"qt{m}") for m in range(2)]
        kt = [acts.tile([128, S], bf16, name=f"kt{m}", tag=f"kt{m}") for m in range(2)]
        vh_sb = acts.tile([128, 16, 260], bf16, name="vh", tag="vh")
        # attn-out as head PAIRS [2 heads x 64 dk = 128 partitions, S]
        outT = [
            acts.tile([128, S], bf16, name=f"outT{p}", tag=f"outT{p}")
            for p in range(2)
        ]

        # ---- constants (DMA order == consume order: K first) ----
        wk_sb = consts.tile([128, 8 * 256], bf16)
        nc.sync.dma_start(out=wk_sb[:], in_=wk[:])
        bk_sb = consts.tile([128, 2], f32)
        nc.sync.dma_start(out=bk_sb[:], in_=bk[:])

        def emit_kq_proj(src_h, wsb, bsb, dst):
            xt = {}
            for half in range(2):
                t = xs.tile([128, 8, S // 2], bf16, name="xt", tag="xt")
                nc.sync.dma_start(out=t[:], in_=src_h[half][:])
                xt[half] = t
            for half in range(2):
                for m in range(2):
                    for scq in range(2):
                        sc = half * 2 + scq
                        ps = ps_small.tile([128, 512], f32, name="ps", tag="ps")
                        for dc in range(8):
                            nc.tensor.matmul(
                                ps[:],
                                lhsT=wsb[
                                    :, dc * 256 + m * 128: dc * 256 + (m + 1) * 128
                                ],
                                rhs=xt[half][:, dc, scq * 512:(scq + 1) * 512],
                                start=(dc == 0),
                                stop=(dc == 7),
                            )
                        # copy+bias+downcast: out = psum + b (per-partition)
                        nc.vector.tensor_scalar_add(
                            dst[m][:, sc * 512:(sc + 1) * 512],
                            ps[:],
                            bsb[:, m:m + 1],
                        )

        def emit_v_proj(wv_sb, bvp_sb):
            xt = {}
            for half in range(2):
                t = xs.tile([128, 8, S // 2], bf16, name="xt", tag="xt")
                nc.sync.dma_start(out=t[:], in_=xv_h[half][:])
                xt[half] = t
            for st in range(16):
                ps = ps_small.tile([128, 512], f32, name="ps", tag="ps")
                for dc in range(8):
                    nc.tensor.matmul(
                        ps[:, :260],
                        lhsT=xt[st // 8][:, dc, (st % 8) * 128:(st % 8 + 1) * 128],
                        rhs=wv_sb[:, dc * 260:(dc + 1) * 260],
                        start=(dc == 0),
                        stop=(dc == 7),
                    )
                # +bias (varies along free dim; bvp_sb is the DMA-broadcast
                # row), writes the ones column too (bvp has 1.0 at h*65+64).
                nc.vector.tensor_add(vh_sb[:, st, :], ps[:, :260], bvp_sb[:])

        if phase >= 1:
            emit_kq_proj(xk_h, wk_sb, bk_sb, kt)
        # remaining constants, in consume order
        wq_sb = consts.tile([128, 8 * 256], bf16, name="wq_sb")
        nc.sync.dma_start(out=wq_sb[:], in_=wq[:])
        bq_sb = consts.tile([128, 2], f32, name="bq_sb")
        nc.sync.dma_start(out=bq_sb[:], in_=bq[:])
        cm_sb = consts.tile([128, 128], bf16, name="cm_sb")
        nc.sync.dma_start(out=cm_sb[:], in_=cmask[:])
        if phase >= 1:
            emit_kq_proj(xq_h, wq_sb, bq_sb, qt)
        wv_sb = consts.tile([128, 8 * 260], bf16, name="wv_sb")
        nc.sync.dma_start(out=wv_sb[:], in_=wv[:])
        bvp_sb = consts.tile([128, 260], f32, name="bvp_sb")
        nc.sync.dma_start(out=bvp_sb[:], in_=bvp[:].to_broadcast((128, 260)))
        if phase >= 2:
            emit_v_proj(wv_sb, bvp_sb)
        wo_sb = consts.tile([128, 2 * 1024], bf16, name="wo_sb")
        nc.sync.dma_start(out=wo_sb[:], in_=wo[:])
        ones_sb = consts.tile([65, 64], bf16)
        nc.vector.memset(ones_sb[:], 1.0)

        # ---- attention ----
        if phase >= 3:
            def trim_c0(qc, kc):
                """Causal column trim for kc-block within q-chunk qc: the
                first 128*jr columns of a diagonal block are fully masked."""
                jr = kc - 4 * qc
                return 128 * jr if jr >= 0 else 0

            def emit_scores_exp(qc, pair, g, exg):
                """Scores matmuls + exp for group g (both heads of pair)."""
                heads = (2 * pair, 2 * pair + 1)
                dg = g - GK * qc  # diagonal subgroup index (>=0 on diagonal)
                for h in heads:
                    hr = slice(64 * (h % 2), 64 * (h % 2) + 64)
                    pss = ps_sc.tile([128, GK, QC], f32, name="pss", tag="pss")
                    for j in range(GK):
                        kc = GK * g + j
                        c0 = trim_c0(qc, kc)
                        nc.tensor.matmul(
                            pss[:, j, c0:],
                            lhsT=kt[pair][hr, kc * 128:(kc + 1) * 128],
                            rhs=qt[pair][hr, qc * QC + c0:(qc + 1) * QC],
                            start=True,
                            stop=True,
                        )
                    ex = exps.tile([128, GK, QC], bf16, name="ex", tag="ex")
                    # full-width exp (a 2-D contiguous AP is much faster on
                    # ScalarE than sliced 3-D ones); the trimmed-away columns
                    # hold stale/garbage PSUM whose exp lands in ex columns
                    # the (equally trimmed) attn@V matmuls never read.
                    nc.scalar.activation(
                        ex[:, :, :],
                        pss[:, :, :],
                        mybir.ActivationFunctionType.Exp,
                    )
                    if dg >= 0:
                        # triangular 128-block of each diagonal kc: keep mask
                        # (runs on the otherwise-idle Pool engine)
                        for j in range(GK):
                            c0 = trim_c0(qc, GK * g + j)
                            nc.gpsimd.tensor_mul(
                                ex[:, j, c0:c0 + 128],
                                ex[:, j, c0:c0 + 128],
                                cm_sb[:],
                            )
                    exg[h] = ex

            def emit_attnv(qc, pair, g, po, last_kc, exg):
                heads = (2 * pair, 2 * pair + 1)
                for h in heads:
                    for j in range(GK):
                        kc = GK * g + j
                        c0 = trim_c0(qc, kc)
                        nc.tensor.matmul(
                            po[h][:, c0:],
                            lhsT=vh_sb[:, kc, h * 65:(h + 1) * 65],
                            rhs=exg[h][:, j, c0:],
                            start=(kc == 0),
                            stop=(kc == last_kc),
                            skip_group_check=True,
                        )

            def emit_pair_tail(qc, pair, po, cur_posb, cur_rcp):
                # denominator reciprocal on ScalarE (exp(-ln x)) + stage
                # attn-out to SBUF bf16 so the po PSUM bank frees.
                for h in (2 * pair, 2 * pair + 1):
                    lg = rcps.tile([65, 512], f32, name="lg", tag="lg", bufs=4)
                    nc.scalar.activation(
                        lg[64:65, :],
                        po[h][64:65, :],
                        mybir.ActivationFunctionType.Ln,
                    )
                    rcp = rcps.tile([65, 512], bf16, name="rcp", tag="rcp", bufs=8)
                    nc.scalar.activation(
                        rcp[64:65, :],
                        lg[64:65, :],
                        mybir.ActivationFunctionType.Exp,
                        scale=-1.0,
                    )
                    posb = rcps.tile([64, 512], bf16, name="posb", tag="posb", bufs=8)
                    with nc.allow_low_precision(reason="attn-out staged bf16"):
                        nc.vector.tensor_copy(posb[:, :], po[h][0:64, :])
                    cur_posb[h] = posb
                    cur_rcp[h] = rcp

            def emit_bc(qc, posb, rcp):
                for h in range(HPC):
                    bc = ps_small.tile([128, 512], f32, name="ps", tag="ps")
                    nc.tensor.matmul(
                        bc[0:64, :],
                        lhsT=ones_sb[64:65, :],
                        rhs=rcp[h][64:65, :],
                        start=True,
                        stop=True,
                    )
                    # DVE can't take two PSUM operands; stage via SBUF, then
                    # the normalize multiply runs on the otherwise-idle GpSimd.
                    bcs = rcps.tile([64, 512], bf16, name="bcs", tag="bcs", bufs=8)
                    nc.vector.tensor_copy(bcs[:, :], bc[0:64, :])
                    nc.gpsimd.tensor_mul(
                        outT[h // 2][64 * (h % 2):64 * (h % 2) + 64,
                                     qc * QC:(qc + 1) * QC],
                        posb[h][:, :],
                        bcs[:, :],
                    )

            def emit_oproj(qc):
                for sti in range(4):
                    st = qc * 4 + sti
                    for ns in range(2):
                        ps = ps_small.tile([128, 512], f32, name="ps", tag="ps")
                        for hp in range(2):
                            nc.tensor.matmul(
                                ps[:],
                                lhsT=outT[hp][:, st * 128:(st + 1) * 128],
                                rhs=wo_sb[
                                    :, hp * 1024 + ns * 512: hp * 1024 + (ns + 1) * 512
                                ],
                                start=(hp == 0),
                                stop=(hp == 1),
                            )
                        ot = osb.tile([128, 512], f16, name="ot", tag="ot")
                        with nc.allow_low_precision(reason="f16 partials"):
                            nc.vector.tensor_copy(ot[:], ps[:])
                        nc.sync.dma_start(
                            out=outp[
                                st * 128:(st + 1) * 128, ns * 512:(ns + 1) * 512
                            ],
                            in_=ot[:],
                        )

            # Per (qc, pair): groups are software-pipelined one deep; the
            # bc/oproj tail of the previous qc fills the PE while the last
            # group's exp drains.
            pending = None  # (qc, posb{h}, rcp{h})
            for qc in range(NQC):
                cur_posb = {}
                cur_rcp = {}
                ngroups = GK * (qc + 1)
                last_kc = 4 * qc + 3
                for pair in range(2):
                    po = {}
                    for h in (2 * pair, 2 * pair + 1):
                        po[h] = ps_av.tile([65, 512], f32, name="po", tag="po")
                    prev = None  # (g, exg)
                    for g in range(ngroups):
                        exg = {}
                        emit_scores_exp(qc, pair, g, exg)
                        if g == ngroups - 1 and pending is not None:
                            # PE filler while exp of the last group drains
                            if pair == 0 and phase >= 4:
                                emit_bc(pending[0], pending[1], pending[2])
                            elif pair == 1 and phase >= 5:
                                emit_oproj(pending[0])
                        if prev is not None:
                            emit_attnv(qc, pair, prev[0], po, last_kc, prev[1])
                        prev = (g, exg)
                    emit_attnv(qc, pair, prev[0], po, last_kc, prev[1])
                    if pending is not None and qc == 0:
                        # no groups to hide behind at qc=0: emit tail work now
                        pass
                    emit_pair_tail(qc, pair, po, cur_posb, cur_rcp)
                pending = (qc, cur_posb, cur_rcp)
            if pending is not None and phase >= 4:
                emit_bc(pending[0], pending[1], pending[2])
                if phase >= 5:
                    emit_oproj(pending[0])

        if phase < 5:
            ot = osb.tile([128, 512], f16, name="ot", tag="ot")
            nc.vector.memset(ot[:], 0.0)
            nc.sync.dma_start(out=outp[0:128, 0:512], in_=ot[:])

    if split:
        _split_multi_waits(nc)
    return nc


_NC_CACHE = None


def _get_nc():
    global _NC_CACHE
    if _NC_CACHE is None:
        _NC_CACHE = _build_nc()
    return _NC_CACHE


def _swizzle_w(wT, block):
    """wT [D, C] -> [128, 8*C] so that out[p, dc*C + j] = wT[dc*128 + p, j]."""
    dcs = wT.shape[0] // 128
    return np.ascontiguousarray(
        wT.reshape(dcs, 128, wT.shape[1]).transpose(1, 0, 2).reshape(128, -1)
    )


def _np_reference(q, k, v, mask, Wq, bq, Wk, bk, Wv, bv, Wo, bo):
    def split_heads(x):
        b, s, _ = x.shape
        return x.reshape(b, s, H, DK).transpose(0, 2, 1, 3)

    qh = split_heads(q @ Wq.T + bq)
    kh = split_heads(k @ Wk.T + bk)
    vh = split_heads(v @ Wv.T + bv)
    scores = np.einsum("bhqd,bhkd->bhqk", qh, kh) / np.sqrt(np.float32(DK))
    scores = np.where(mask, np.float32(-1e9), scores)
    scores = scores - scores.max(axis=-1, keepdims=True)
    e = np.exp(scores)
    attn = e / e.sum(axis=-1, keepdims=True)
    out = np.einsum("bhqk,bhkd->bhqd", attn, vh)
    out = out.transpose(0, 2, 1, 3).reshape(q.shape[0], -1, D)
    return (out @ Wo.T + bo).astype(np.float32)


def kernel(q, k, v, mask, Wq, bq, Wk, bk, Wv, bv, Wo, bo):
    import ml_dtypes

    bf16 = ml_dtypes.bfloat16

    q = np.asarray(q, np.float32)
    k = np.asarray(k, np.float32)
    v = np.asarray(v, np.float32)
    mask = np.asarray(mask, bool)
    Wq = np.asarray(Wq, np.float32)
    bq = np.asarray(bq, np.float32)
    Wk = np.asarray(Wk, np.float32)
    bk = np.asarray(bk, np.float32)
    Wv = np.asarray(Wv, np.float32)
    bv = np.asarray(bv, np.float32)
    Wo = np.asarray(Wo, np.float32)
    bo = np.asarray(bo, np.float32)

    causal = np.triu(np.ones((S, S), dtype=bool), k=1)
    if not np.array_equal(mask.reshape(S, S), causal):
        return _np_reference(q, k, v, mask, Wq, bq, Wk, bk, Wv, bv, Wo, bo)

    _install_ntff_hook()
    from concourse.bass_utils import run_bass_kernel_spmd

    nc = _get_nc()

    # triangular keep-mask for the partial 128x128 diagonal blocks:
    # keep iff kk <= qq
    kk = np.arange(128)[:, None]
    qq = np.arange(128)[None, :]
    cm = (kk <= qq).astype(bf16)  # [128, 128]

    # x.T [D, S] -> per half [128, 8, 1024] with x_h[p, dc, s] =
    # xT[dc*128 + p, half*1024 + s]; contiguous 16KB per partition.
    xT = {}
    for name, x in (("q", q), ("k", k), ("v", v)):
        per_b = []
        for b in range(B):
            xt = x[b].T.astype(bf16).reshape(8, 128, 2048)
            per_b.append(
                [
                    np.ascontiguousarray(
                        xt[:, :, hf * 1024:(hf + 1) * 1024].transpose(1, 0, 2)
                    )
                    for hf in range(2)
                ]
            )
        xT[name] = per_b

    in_maps = []
    for c in range(N_CORES):
        b = c // 4
        g = c % 4
        hs = slice(g * HPC * DK, (g + 1) * HPC * DK)  # 256 rows of W, cols of Wo
        wq_c = _swizzle_w((SCALE * Wq[hs]).T.astype(bf16), 256)
        wk_c = _swizzle_w(Wk[hs].T.astype(bf16), 256)
        # V' with a zero weight column at h*65+64 (ones come via bias row)
        wvT = Wv[hs].T  # [1024, 256]
        wvp = np.zeros((D, 260), np.float32)
        for h in range(HPC):
            wvp[:, h * 65:h * 65 + 64] = wvT[:, h * 64:(h + 1) * 64]
        wv_c = _swizzle_w(wvp.astype(bf16), 260)
        # wo: (Wo.T)[hs, :] [256, 1024] -> head-pair blocks [128, 2*1024]
        woT = np.ascontiguousarray(Wo[:, hs].T)
        wo_c = np.ascontiguousarray(
            woT.reshape(2, 128, 1024).transpose(1, 0, 2).reshape(128, 2048)
        ).astype(bf16)
        bq_c = np.ascontiguousarray(
            (SCALE * bq[hs]).reshape(2, 128).T.astype(np.float32)
        )
        bk_c = np.ascontiguousarray(bk[hs].reshape(2, 128).T.astype(np.float32))
        bvp_c = np.zeros((1, 260), np.float32)
        for h in range(HPC):
            bvp_c[0, h * 65:h * 65 + 64] = bv[hs][h * 64:(h + 1) * 64]
            bvp_c[0, h * 65 + 64] = 1.0
        in_maps.append(
            {
                "xq0": xT["q"][b][0],
                "xq1": xT["q"][b][1],
                "xk0": xT["k"][b][0],
                "xk1": xT["k"][b][1],
                "xv0": xT["v"][b][0],
                "xv1": xT["v"][b][1],
                "wq": wq_c,
                "wk": wk_c,
                "wv": wv_c,
                "wo": wo_c,
                "bq": bq_c,
                "bk": bk_c,
                "bvp": bvp_c,
                "cmask": cm,
            }
        )

    trace = bool(os.environ.get("BASSMHA_TRACE"))
    res = run_bass_kernel_spmd(nc, in_maps, list(range(N_CORES)), trace=trace)
    kernel._last_exec_ns = res.exec_time_ns
    kernel._last_mean_exec_ns = res.mean_exec_time_ns

    out = np.zeros((B, S, D), np.float64)
    for c in range(N_CORES):
        out[c // 4] += res.results[c]["outp"].astype(np.float64)
    out += bo.astype(np.float64)
    return out.astype(np.float32)
